# revision 17
# baseline (speedup 1.0000x reference)
"""BiDAF-style attention kernel for Trainium2, 8-core data-parallel over batch.

Problem (per batch b):
  sim[c,q] = ctx[c]@w_c + qry[q]@w_q + sum_h ctx[c,h] w_m[h] qry[q,h] + att_b
  alpha = softmax_q(sim);        a[c] = sum_q alpha[c,q] qry[q]
  beta  = softmax_c(max_q sim);  bv   = sum_c beta[c] ctx[c]
  out = [ctx | a | ctx*a | ctx*bv]          (C, 4H)

Key algebra:
  - ctx@w_c (cvec) is constant along q -> cancels in the alpha softmax and in
    a; it only shifts the beta logits. So sim' = sim - cvec is computed on the
    PE and cvec enters only as a tiny per-c weight exp(cvec) on the beta path.
  - att_b is a global constant -> cancels everywhere; dropped entirely.
  - No max subtraction inside softmax: logits are O(10), exp is safe in f32,
    and the shift cancels exactly.
  - max_q exp(sim') = exp(max_q sim'), so the beta max is the rowmax of the
    already-computed exp values.

Layout: sim' is built TRANSPOSED, simT [q=128 part, c=1024 free], so the main
matmuls run N=512/257 with fp32r (1 cycle/row, single pass):
  simT = qTs_r^T @ ctxT_r   (qTs = w_m * qT); qvec = qry@w_q is a
  per-partition scalar in this layout and enters via the exp bias for free.
  expsimT = exp(simT + qvec) written as f32r -> directly the lhsT of the
  a-matmul: [a | S] = expsimT^T @ [qry | 1], S = alpha normalizer from the
  ones column. The beta max comes from PE-transposing expsimT tiles back to
  [c,q] and DVE row-maxing them straight out of PSUM.
"""

import numpy as np

import concourse.bass as bass
import concourse.tile as tile
from concourse import mybir
from concourse.bass_utils import run_bass_kernel_spmd
from concourse.masks import make_identity

B, C, Q, H = 64, 1024, 128, 256
NCORES = 8
BL = B // NCORES          # batches per core
CT = C // 128             # context row-tiles per batch
F32 = mybir.dt.float32
F32R = mybir.dt.float32r


def split_waits(nc, max_waits=1):
    """walrus codegen in this container rejects >1 sem wait per instruction;
    move excess waits onto same-engine NoOps inserted just before."""
    n_new = 0
    for f in nc.m.functions:
        for blk in f.blocks:
            out = []
            for ins in blk.instructions:
                waits = list(ins.sync_info.on_wait) if ins.sync_info else []
                if len(waits) > max_waits:
                    extra, keep = waits[:-max_waits], waits[-max_waits:]
                    for j in range(0, len(extra), max_waits):
                        nop = mybir.InstNoOp(name=f"I-wsplit-{n_new}", ins=[], outs=[])
                        n_new += 1
                        nop.engine = ins.engine
                        nop.sync_info = mybir.SyncInfo(
                            on_wait=list(extra[j : j + max_waits]), on_update=[]
                        )
                        out.append(nop)
                    ins.sync_info.on_wait = list(keep)
                out.append(ins)
            blk.instructions = out
    return n_new


def build():
    nc = bass.Bass()
    ctx_d = nc.dram_tensor("context", [BL, C, H], F32, kind="ExternalInput")
    q_d = nc.dram_tensor("query", [BL, Q, H], F32, kind="ExternalInput")
    w_d = nc.dram_tensor("att_w", [3 * H], F32, kind="ExternalInput")
    b_d = nc.dram_tensor("att_b", [1], F32, kind="ExternalInput")
    out_d = nc.dram_tensor("out", [BL, C, 4 * H], F32, kind="ExternalOutput")

    X = mybir.AxisListType.X
    EXP = mybir.ActivationFunctionType.Exp

    with tile.TileContext(nc) as tc:
        from contextlib import ExitStack

        with ExitStack() as ctx:
            consts = ctx.enter_context(tc.tile_pool(name="consts", bufs=1))
            ctxp = ctx.enter_context(tc.tile_pool(name="ctx", bufs=3))
            ctxTp = ctx.enter_context(tc.tile_pool(name="ctxT", bufs=3))
            qp = ctx.enter_context(tc.tile_pool(name="qp", bufs=3))
            esp = ctx.enter_context(tc.tile_pool(name="es", bufs=3))
            stagp = ctx.enter_context(tc.tile_pool(name="stag", bufs=2))
            cbvp = ctx.enter_context(tc.tile_pool(name="cbv", bufs=2))
            smallp = ctx.enter_context(tc.tile_pool(name="small", bufs=8))
            ps_sim = ctx.enter_context(tc.tile_pool(name="ps_sim", bufs=2, space="PSUM"))
            ps_tp = ctx.enter_context(tc.tile_pool(name="ps_tp", bufs=2, space="PSUM"))
            ps_a = ctx.enter_context(tc.tile_pool(name="ps_a", bufs=2, space="PSUM"))
            ps_sm = ctx.enter_context(tc.tile_pool(name="ps_sm", bufs=1, space="PSUM"))

            ident = consts.tile([128, 128], F32)
            make_identity(nc, ident[:, :])
            ident_r = consts.tile([128, 128], F32R)
            nc.vector.tensor_copy(ident_r[:, :], ident[:, :])
            ones_col = consts.tile([128, 1], F32)
            nc.vector.memset(ones_col[:, :], 1.0)
            ones_row = consts.tile([1, 128], F32)
            nc.vector.memset(ones_row[:, :], 1.0)
            ones_row_r = consts.tile([1, 128], F32R)
            nc.vector.tensor_copy(ones_row_r[:, :], ones_row[:, :])
            # att_w as 6 columns: [w_c h0|h1, w_q h0|h1, w_m h0|h1]
            wcols = consts.tile([128, 6], F32)
            nc.gpsimd.dma_start(
                out=wcols[:, :],
                in_=bass.AP(tensor=w_d, offset=0, ap=[[1, 128], [128, 6]]),
            )
            wc_r = consts.tile([128, 2], F32R)
            nc.vector.tensor_copy(wc_r[:, :], wcols[:, 0:2])
            # w_q broadcast across partitions for the qvec row-reduction
            wqb = consts.tile([128, H], F32)
            nc.gpsimd.dma_start(
                out=wqb[:, :],
                in_=bass.AP(tensor=w_d, offset=H, ap=[[0, 128], [1, H]]),
            )

            for b in range(BL):
                ctx_sb = ctxp.tile([128, CT, H], F32)
                nc.scalar.dma_start(
                    out=ctx_sb[:, :, :],
                    in_=ctx_d[b].rearrange("(ct p) h -> p ct h", p=128),
                )
                q_sb = qp.tile([128, H], F32)
                nc.scalar.dma_start(out=q_sb[:, :], in_=q_d[b])
                nc.sync.dma_start(
                    out=out_d[b, :, 0:H].rearrange("(ct p) h -> p ct h", p=128),
                    in_=ctx_sb[:, :, :],
                )

                # qT scaled by w_m -> lhsT of the simT matmul (f32r)
                qTs_r = qp.tile([128, 2, 128], F32R)
                for ht in range(2):
                    tp = ps_tp.tile([128, 128], F32, tag="tp")
                    nc.tensor.transpose(
                        tp[:, :], q_sb[:, ht * 128 : (ht + 1) * 128], ident[:, :]
                    )
                    nc.vector.tensor_scalar_mul(
                        qTs_r[:, ht, :], tp[:, :], wcols[:, 4 + ht : 5 + ht]
                    )

                # qvec[q] = qry[q] @ w_q as a column (q = partition dim)
                scr = qp.tile([128, H], F32)
                qvec_col = smallp.tile([128, 1], F32)
                nc.vector.tensor_mul(scr[:, :], q_sb[:, :], wqb[:, :])
                nc.vector.reduce_sum(qvec_col[:, :], scr[:, :], axis=X)

                # rhs of the a-matmul: [qry | 1] rounded to f32r
                qaug_r = qp.tile([128, H + 2 + 128], F32R)
                nc.vector.tensor_copy(qaug_r[:, 0:H], q_sb[:, :])
                nc.vector.tensor_copy(qaug_r[:, H : H + 1], ones_col[:, :])
                nc.vector.tensor_copy(qaug_r[:, H + 1 : H + 2], ones_col[:, :])
                nc.vector.tensor_copy(qaug_r[:, H + 2 : H + 2 + 128], ident_r[:, :])

                # rounded ctx once; f32r transposes (1.5 cyc/row) for ctxT
                ctx_r = ctxp.tile([128, CT, H + 2], F32R)
                for ct in range(CT):
                    nc.scalar.copy(ctx_r[:, ct, 0:H], ctx_sb[:, ct, :])
                ones_b = bass.AP(
                    tensor=ones_col.tensor,
                    offset=ones_col[:, :].offset,
                    ap=[ones_col[:, :].ap[0], [0, CT], [0, 2]],
                )
                nc.vector.tensor_copy(ctx_r[:, :, H : H + 2], ones_b)
                ctxT_r = ctxTp.tile([128, 2, C], F32R)
                for ht in range(2):
                    for ct in range(CT):
                        tp = ps_tp.tile([128, 128], F32R, tag="tp")
                        nc.tensor.matmul(
                            tp[:, :],
                            lhsT=ctx_r[:, ct, ht * 128 : (ht + 1) * 128],
                            rhs=ident_r[:, :],
                            start=True,
                            stop=True,
                            is_transpose=True,
                        )
                        nc.scalar.copy(
                            ctxT_r[:, ht, ct * 128 : (ct + 1) * 128], tp[:, :].bitcast(F32)
                        )

                # cvec row: w_c^T @ ctxT (fp32r, N=512) -> rounded SBUF row
                cvec_r = smallp.tile([1, C], F32R, tag="cvec")
                for ch in range(2):
                    cvr = ps_sm.tile([1, 512], F32, tag="sm")
                    for ht in range(2):
                        nc.tensor.matmul(
                            cvr[:, :],
                            lhsT=wc_r[:, ht : ht + 1],
                            rhs=ctxT_r[:, ht, ch * 512 : (ch + 1) * 512],
                            start=(ht == 0),
                            stop=(ht == 1),
                        )
                    nc.scalar.copy(cvec_r[:, ch * 512 : (ch + 1) * 512], cvr[:, :])

                # simT[q, c] = qTs^T @ ctxT + 1 (x) cvec   (fp32r, N=512)
                es_r = esp.tile([128, C], F32R)
                simTs = []
                for _ch in range(2):
                    simT_t = ps_sim.tile([128, 512], F32, tag="sim")
                    simTs.append(simT_t)
                for ht in range(2):
                    for ch in range(2):
                        nc.tensor.matmul(
                            simTs[ch][:, :],
                            lhsT=qTs_r[:, ht, :],
                            rhs=ctxT_r[:, ht, ch * 512 : (ch + 1) * 512],
                            start=(ht == 0),
                            stop=False,
                        )
                for ch in range(2):
                    nc.tensor.matmul(
                        simTs[ch][:, :],
                        lhsT=ones_row_r[:, :],
                        rhs=cvec_r[:, ch * 512 : (ch + 1) * 512],
                        start=False,
                        stop=True,
                    )
                for ch in range(2):
                    nc.scalar.activation(
                        out=es_r[:, ch * 512 : (ch + 1) * 512],
                        in_=simTs[ch][:, :],
                        func=EXP,
                        bias=qvec_col[:, 0:1],
                        scale=1.0,
                    )

                M8w_r = smallp.tile([128, CT], F32R)
                bv_ps = ps_sm.tile([1, H + 2], F32, tag="bv")
                actxa = stagp.tile([128, CT, 2, H], F32)
                cbv8 = cbvp.tile([128, CT, H], F32)
                for ct in range(CT):
                    # one matmul: [a_unnorm | S | S | es^T]  (transpose via identity block)
                    af = ps_a.tile([128, H + 2 + 128], F32, tag="a")
                    nc.tensor.matmul(
                        af[:, :],
                        lhsT=es_r[:, ct * 128 : (ct + 1) * 128],
                        rhs=qaug_r[:, :],
                        start=True,
                        stop=True,
                    )
                    rS = smallp.tile([128, 1], F32)
                    nc.vector.reciprocal(rS[:, :], af[:, H : H + 1])
                    nc.vector.tensor_scalar_mul(actxa[:, ct, 0, :], af[:, 0:H], rS[:, :])
                    nc.gpsimd.tensor_mul(
                        actxa[:, ct, 1, :], ctx_sb[:, ct, :], actxa[:, ct, 0, :]
                    )
                    nc.vector.reduce_max(
                        M8w_r[:, ct : ct + 1], af[:, H + 2 : H + 2 + 128], axis=X
                    )
                    nc.tensor.matmul(
                        bv_ps[:, :],
                        lhsT=M8w_r[:, ct : ct + 1],
                        rhs=ctx_r[:, ct, :],
                        start=(ct == 0),
                        stop=(ct == CT - 1),
                        skip_group_check=True,
                    )
                    nc.sync.dma_start(
                        out=out_d[b, ct * 128 : (ct + 1) * 128, H : 3 * H],
                        in_=actxa[:, ct, :, :],
                    )

                rSb = smallp.tile([1, 1], F32)
                nc.vector.reciprocal(rSb[:, :], bv_ps[:, H : H + 1])
                bv_r = smallp.tile([1, H], F32R)
                nc.vector.tensor_scalar_mul(bv_r[:, :], bv_ps[:, 0:H], rSb[:, :])
                bb_ps = ps_a.tile([128, H + 2], F32, tag="a")
                nc.tensor.matmul(
                    bb_ps[:, 0:H],
                    lhsT=ones_row_r[:, :],
                    rhs=bv_r[:, :],
                    start=True,
                    stop=True,
                )
                bb_bcast = bass.AP(
                    tensor=bb_ps.tensor,
                    offset=bb_ps[:, 0:H].offset,
                    ap=[bb_ps[:, 0:H].ap[0], [0, CT], [1, H]],
                )
                nc.vector.tensor_mul(cbv8[:, :, :], ctx_sb[:, :, :], bb_bcast)
                nc.sync.dma_start(
                    out=out_d[b, :, 3 * H : 4 * H].rearrange("(ct p) h -> p ct h", p=128),
                    in_=cbv8[:, :, :],
                )

    split_waits(nc)
    return nc


_NC = None
LAST_RESULT = None


def kernel(_trace=False, **inputs):
    global _NC, LAST_RESULT
    if _NC is None:
        _NC = build()
    context = np.ascontiguousarray(np.asarray(inputs["context"], dtype=np.float32))
    query = np.ascontiguousarray(np.asarray(inputs["query"], dtype=np.float32))
    att_w = np.ascontiguousarray(np.asarray(inputs["att_w"], dtype=np.float32))
    att_b = np.asarray(inputs["att_b"], dtype=np.float32).reshape(1)
    in_maps = [
        {
            "context": np.ascontiguousarray(context[i * BL : (i + 1) * BL]),
            "query": np.ascontiguousarray(query[i * BL : (i + 1) * BL]),
            "att_w": att_w,
            "att_b": att_b,
        }
        for i in range(NCORES)
    ]
    res = run_bass_kernel_spmd(
        _NC, in_maps, core_ids=list(range(NCORES)), trace=_trace
    )
    LAST_RESULT = res
    return np.concatenate([r["out"] for r in res.results], axis=0)


# revision 19
# speedup vs baseline: 1.0330x; 1.0330x over previous
"""BiDAF-style attention kernel for Trainium2, 8-core data-parallel over batch.

Problem (per batch b):
  sim[c,q] = ctx[c]@w_c + qry[q]@w_q + sum_h ctx[c,h] w_m[h] qry[q,h] + att_b
  alpha = softmax_q(sim);        a[c] = sum_q alpha[c,q] qry[q]
  beta  = softmax_c(max_q sim);  bv   = sum_c beta[c] ctx[c]
  out = [ctx | a | ctx*a | ctx*bv]          (C, 4H)

Key algebra:
  - ctx@w_c (cvec) is constant along q -> cancels in the alpha softmax and in
    a; it only shifts the beta logits. So sim' = sim - cvec is computed on the
    PE and cvec enters only as a tiny per-c weight exp(cvec) on the beta path.
  - att_b is a global constant -> cancels everywhere; dropped entirely.
  - No max subtraction inside softmax: logits are O(10), exp is safe in f32,
    and the shift cancels exactly.
  - max_q exp(sim') = exp(max_q sim'), so the beta max is the rowmax of the
    already-computed exp values.

Layout: sim' is built TRANSPOSED, simT [q=128 part, c=1024 free], so the main
matmuls run N=512/257 with fp32r (1 cycle/row, single pass):
  simT = qTs_r^T @ ctxT_r   (qTs = w_m * qT); qvec = qry@w_q is a
  per-partition scalar in this layout and enters via the exp bias for free.
  expsimT = exp(simT + qvec) written as f32r -> directly the lhsT of the
  a-matmul: [a | S] = expsimT^T @ [qry | 1], S = alpha normalizer from the
  ones column. The beta max comes from PE-transposing expsimT tiles back to
  [c,q] and DVE row-maxing them straight out of PSUM.
"""

import numpy as np

import concourse.bass as bass
import concourse.tile as tile
from concourse import mybir
from concourse.bass_utils import run_bass_kernel_spmd
from concourse.masks import make_identity

B, C, Q, H = 64, 1024, 128, 256
NCORES = 8
BL = B // NCORES          # batches per core
CT = C // 128             # context row-tiles per batch
F32 = mybir.dt.float32
F32R = mybir.dt.float32r


def split_waits(nc, max_waits=1):
    """walrus codegen in this container rejects >1 sem wait per instruction;
    move excess waits onto same-engine NoOps inserted just before."""
    n_new = 0
    for f in nc.m.functions:
        for blk in f.blocks:
            out = []
            for ins in blk.instructions:
                waits = list(ins.sync_info.on_wait) if ins.sync_info else []
                if len(waits) > max_waits:
                    extra, keep = waits[:-max_waits], waits[-max_waits:]
                    for j in range(0, len(extra), max_waits):
                        nop = mybir.InstNoOp(name=f"I-wsplit-{n_new}", ins=[], outs=[])
                        n_new += 1
                        nop.engine = ins.engine
                        nop.sync_info = mybir.SyncInfo(
                            on_wait=list(extra[j : j + max_waits]), on_update=[]
                        )
                        out.append(nop)
                    ins.sync_info.on_wait = list(keep)
                out.append(ins)
            blk.instructions = out
    return n_new


def build():
    nc = bass.Bass()
    ctx_d = nc.dram_tensor("context", [BL, C, H], F32, kind="ExternalInput")
    q_d = nc.dram_tensor("query", [BL, Q, H], F32, kind="ExternalInput")
    w_d = nc.dram_tensor("att_w", [3 * H], F32, kind="ExternalInput")
    b_d = nc.dram_tensor("att_b", [1], F32, kind="ExternalInput")
    out_d = nc.dram_tensor("out", [BL, C, 4 * H], F32, kind="ExternalOutput")

    X = mybir.AxisListType.X
    EXP = mybir.ActivationFunctionType.Exp

    with tile.TileContext(nc) as tc:
        from contextlib import ExitStack

        with ExitStack() as ctx:
            consts = ctx.enter_context(tc.tile_pool(name="consts", bufs=1))
            ctxp = ctx.enter_context(tc.tile_pool(name="ctx", bufs=3))
            ctxTp = ctx.enter_context(tc.tile_pool(name="ctxT", bufs=3))
            qp = ctx.enter_context(tc.tile_pool(name="qp", bufs=3))
            esp = ctx.enter_context(tc.tile_pool(name="es", bufs=3))
            stagp = ctx.enter_context(tc.tile_pool(name="stag", bufs=2))
            cbvp = ctx.enter_context(tc.tile_pool(name="cbv", bufs=2))
            smallp = ctx.enter_context(tc.tile_pool(name="small", bufs=8))
            ps_sim = ctx.enter_context(tc.tile_pool(name="ps_sim", bufs=1, space="PSUM"))
            ps_tp = ctx.enter_context(tc.tile_pool(name="ps_tp", bufs=2, space="PSUM"))
            ps_a = ctx.enter_context(tc.tile_pool(name="ps_a", bufs=3, space="PSUM"))
            ps_sm = ctx.enter_context(tc.tile_pool(name="ps_sm", bufs=1, space="PSUM"))

            ident = consts.tile([128, 128], F32)
            make_identity(nc, ident[:, :])
            ident_r = consts.tile([128, 128], F32R)
            nc.vector.tensor_copy(ident_r[:, :], ident[:, :])
            ones_col = consts.tile([128, 1], F32)
            nc.vector.memset(ones_col[:, :], 1.0)
            ones_row = consts.tile([1, 128], F32)
            nc.vector.memset(ones_row[:, :], 1.0)
            ones_row_r = consts.tile([1, 128], F32R)
            nc.vector.tensor_copy(ones_row_r[:, :], ones_row[:, :])
            # att_w as 6 columns: [w_c h0|h1, w_q h0|h1, w_m h0|h1]
            wcols = consts.tile([128, 6], F32)
            nc.gpsimd.dma_start(
                out=wcols[:, :],
                in_=bass.AP(tensor=w_d, offset=0, ap=[[1, 128], [128, 6]]),
            )
            wc_r = consts.tile([128, 2], F32R)
            nc.vector.tensor_copy(wc_r[:, :], wcols[:, 0:2])
            # w_q broadcast across partitions for the qvec row-reduction
            wqb = consts.tile([128, H], F32)
            nc.gpsimd.dma_start(
                out=wqb[:, :],
                in_=bass.AP(tensor=w_d, offset=H, ap=[[0, 128], [1, H]]),
            )

            for b in range(BL):
                ctx_sb = ctxp.tile([128, CT, H], F32)
                nc.scalar.dma_start(
                    out=ctx_sb[:, :, :],
                    in_=ctx_d[b].rearrange("(ct p) h -> p ct h", p=128),
                )
                q_sb = qp.tile([128, H], F32)
                nc.scalar.dma_start(out=q_sb[:, :], in_=q_d[b])
                nc.sync.dma_start(
                    out=out_d[b, :, 0:H].rearrange("(ct p) h -> p ct h", p=128),
                    in_=ctx_sb[:, :, :],
                )

                # qT scaled by w_m -> lhsT of the simT matmul (f32r)
                qTs_r = qp.tile([128, 2, 128], F32R)
                for ht in range(2):
                    tp = ps_tp.tile([128, 128], F32, tag="tp")
                    nc.tensor.transpose(
                        tp[:, :], q_sb[:, ht * 128 : (ht + 1) * 128], ident[:, :]
                    )
                    nc.vector.tensor_scalar_mul(
                        qTs_r[:, ht, :], tp[:, :], wcols[:, 4 + ht : 5 + ht]
                    )

                # qvec[q] = qry[q] @ w_q as a column (q = partition dim)
                scr = qp.tile([128, H], F32)
                qvec_col = smallp.tile([128, 1], F32)
                nc.vector.tensor_mul(scr[:, :], q_sb[:, :], wqb[:, :])
                nc.vector.reduce_sum(qvec_col[:, :], scr[:, :], axis=X)

                # rhs of the a-matmul: [qry | 1] rounded to f32r
                qaug_r = qp.tile([128, H + 2 + 128], F32R)
                nc.vector.tensor_copy(qaug_r[:, 0:H], q_sb[:, :])
                nc.vector.tensor_copy(qaug_r[:, H : H + 1], ones_col[:, :])
                nc.vector.tensor_copy(qaug_r[:, H + 1 : H + 2], ones_col[:, :])
                nc.vector.tensor_copy(qaug_r[:, H + 2 : H + 2 + 128], ident_r[:, :])

                # rounded ctx once; f32r transposes (1.5 cyc/row) for ctxT
                ctx_r = ctxp.tile([128, CT, H + 2], F32R)
                for ct in range(CT):
                    nc.scalar.copy(ctx_r[:, ct, 0:H], ctx_sb[:, ct, :])
                ones_b = bass.AP(
                    tensor=ones_col.tensor,
                    offset=ones_col[:, :].offset,
                    ap=[ones_col[:, :].ap[0], [0, CT], [0, 2]],
                )
                nc.vector.tensor_copy(ctx_r[:, :, H : H + 2], ones_b)
                ctxT_r = ctxTp.tile([128, 2, C], F32R)
                for ht in range(2):
                    for ct in range(CT):
                        tp = ps_tp.tile([128, 128], F32R, tag="tp")
                        nc.tensor.matmul(
                            tp[:, :],
                            lhsT=ctx_r[:, ct, ht * 128 : (ht + 1) * 128],
                            rhs=ident_r[:, :],
                            start=True,
                            stop=True,
                            is_transpose=True,
                        )
                        if (ht * CT + ct) % 2 == 0:
                            nc.scalar.copy(
                                ctxT_r[:, ht, ct * 128 : (ct + 1) * 128],
                                tp[:, :].bitcast(F32),
                            )
                        else:
                            nc.vector.tensor_copy(
                                ctxT_r[:, ht, ct * 128 : (ct + 1) * 128], tp[:, :]
                            )

                # cvec row: w_c^T @ ctxT (fp32r, N=512) -> rounded SBUF row
                cvec_r = smallp.tile([1, C], F32R, tag="cvec")
                for ch in range(2):
                    cvr = ps_tp.tile([1, 512], F32, tag="tp")
                    for ht in range(2):
                        nc.tensor.matmul(
                            cvr[:, :],
                            lhsT=wc_r[:, ht : ht + 1],
                            rhs=ctxT_r[:, ht, ch * 512 : (ch + 1) * 512],
                            start=(ht == 0),
                            stop=(ht == 1),
                        )
                    nc.scalar.copy(cvec_r[:, ch * 512 : (ch + 1) * 512], cvr[:, :])

                # simT[q, c] = qTs^T @ ctxT + 1 (x) cvec   (fp32r, N=512)
                es_r = esp.tile([128, C], F32R)
                simT_a = ps_sim.tile([128, 512], F32, tag="sim0")
                simT_b = ps_sim.tile([128, 512], F32, tag="sim1")
                simTs = [simT_a, simT_b]
                for ht in range(2):
                    for ch in range(2):
                        nc.tensor.matmul(
                            simTs[ch][:, :],
                            lhsT=qTs_r[:, ht, :],
                            rhs=ctxT_r[:, ht, ch * 512 : (ch + 1) * 512],
                            start=(ht == 0),
                            stop=False,
                        )
                for ch in range(2):
                    nc.tensor.matmul(
                        simTs[ch][:, :],
                        lhsT=ones_row_r[:, :],
                        rhs=cvec_r[:, ch * 512 : (ch + 1) * 512],
                        start=False,
                        stop=True,
                    )
                for ch in range(2):
                    nc.scalar.activation(
                        out=es_r[:, ch * 512 : (ch + 1) * 512],
                        in_=simTs[ch][:, :],
                        func=EXP,
                        bias=qvec_col[:, 0:1],
                        scale=1.0,
                    )

                M8w_r = smallp.tile([128, CT], F32R)
                bv_ps = ps_sm.tile([1, H + 2], F32, tag="bv")
                actxa = stagp.tile([128, CT, 2, H], F32)
                cbv8 = cbvp.tile([128, CT, H], F32)
                for ct in range(CT):
                    # one matmul: [a_unnorm | S | S | es^T]  (transpose via identity block)
                    af = ps_a.tile([128, H + 2 + 128], F32, tag="a")
                    nc.tensor.matmul(
                        af[:, :],
                        lhsT=es_r[:, ct * 128 : (ct + 1) * 128],
                        rhs=qaug_r[:, :],
                        start=True,
                        stop=True,
                    )
                    rS = smallp.tile([128, 1], F32)
                    nc.vector.reciprocal(rS[:, :], af[:, H : H + 1])
                    nc.vector.tensor_scalar_mul(actxa[:, ct, 0, :], af[:, 0:H], rS[:, :])
                    nc.gpsimd.tensor_mul(
                        actxa[:, ct, 1, :], ctx_sb[:, ct, :], actxa[:, ct, 0, :]
                    )
                    nc.vector.reduce_max(
                        M8w_r[:, ct : ct + 1], af[:, H + 2 : H + 2 + 128], axis=X
                    )
                    nc.tensor.matmul(
                        bv_ps[:, :],
                        lhsT=M8w_r[:, ct : ct + 1],
                        rhs=ctx_r[:, ct, :],
                        start=(ct == 0),
                        stop=(ct == CT - 1),
                        skip_group_check=True,
                    )


                rSb = smallp.tile([1, 1], F32)
                nc.vector.reciprocal(rSb[:, :], bv_ps[:, H : H + 1])
                bv_r = smallp.tile([1, H], F32R)
                nc.vector.tensor_scalar_mul(bv_r[:, :], bv_ps[:, 0:H], rSb[:, :])
                nc.sync.dma_start(
                    out=out_d[b, :, H : 3 * H].rearrange("(ct p) h -> p ct h", p=128),
                    in_=actxa[:, :, :, :],
                )
                bb_ps = ps_a.tile([128, H + 2 + 128], F32, tag="a")
                nc.tensor.matmul(
                    bb_ps[:, 0:H],
                    lhsT=ones_row_r[:, :],
                    rhs=bv_r[:, :],
                    start=True,
                    stop=True,
                )
                bb_bcast = bass.AP(
                    tensor=bb_ps.tensor,
                    offset=bb_ps[:, 0:H].offset,
                    ap=[bb_ps[:, 0:H].ap[0], [0, CT], [1, H]],
                )
                nc.vector.tensor_mul(cbv8[:, :, :], ctx_sb[:, :, :], bb_bcast)
                nc.sync.dma_start(
                    out=out_d[b, :, 3 * H : 4 * H].rearrange("(ct p) h -> p ct h", p=128),
                    in_=cbv8[:, :, :],
                )

    split_waits(nc)
    return nc


_NC = None
LAST_RESULT = None


def kernel(_trace=False, **inputs):
    global _NC, LAST_RESULT
    if _NC is None:
        _NC = build()
    context = np.ascontiguousarray(np.asarray(inputs["context"], dtype=np.float32))
    query = np.ascontiguousarray(np.asarray(inputs["query"], dtype=np.float32))
    att_w = np.ascontiguousarray(np.asarray(inputs["att_w"], dtype=np.float32))
    att_b = np.asarray(inputs["att_b"], dtype=np.float32).reshape(1)
    in_maps = [
        {
            "context": np.ascontiguousarray(context[i * BL : (i + 1) * BL]),
            "query": np.ascontiguousarray(query[i * BL : (i + 1) * BL]),
            "att_w": att_w,
            "att_b": att_b,
        }
        for i in range(NCORES)
    ]
    res = run_bass_kernel_spmd(
        _NC, in_maps, core_ids=list(range(NCORES)), trace=_trace
    )
    LAST_RESULT = res
    return np.concatenate([r["out"] for r in res.results], axis=0)
